# revision 2
# baseline (speedup 1.0000x reference)
"""Trainium2 Bass kernel for LocalWindowAttention.

Reference computation (see problem):
  x [B=2, S=8192, D=1024] -> q,k,v = x @ {wq,wk,wv}.T ; RoPE(q,k) with
  positions arange(16) per 16-token window; non-causal softmax attention
  within each window; out = attn @ wo.T.

Strategy: data-parallel over tokens (8 cores x 2048 tokens; windows of 16
never cross shard boundaries).  Per core, two passes over 128-token tiles:

  pass 1: xT tiles -> Q,K,V projections (fp32r matmuls, N=512) -> RoPE on
          Q,K (DVE) -> PE-transpose Q,K to [d, tok] -> per head, scoresT =
          KT_h^T @ QT_h (block of 8 windows; off-window entries masked) ->
          exp on ACT -> multiplicative block mask -> PV matmul with a ones
          column appended to V so the softmax denominator Z lands in the
          same PSUM tile -> normalize via per-partition tensor_scalar ->
          PE-transpose attn to [d, tok] -> DRAM (fp32r).
  pass 2: attnT tiles @ woT (fp32r) -> out rows.

wo reuses wq's SBUF slot (same tile pool tag), keeping resident weights at
12MB so everything fits in SBUF.
"""

import os
import sys

import numpy as np

for _p in ("/root/.axon_site/_ro/trn_rl_repo", "/opt/trn_rl_repo"):
    if os.path.isdir(_p) and _p not in sys.path:
        sys.path.append(_p)

import concourse.bass as bass
import concourse.tile as tile
from concourse import bacc, mybir
from concourse.bass_utils import run_bass_kernel_spmd

F32 = mybir.dt.float32
F32R = mybir.dt.float32r
AF = mybir.ActivationFunctionType

DIM = 1024
NHEADS = 16
HDIM = 64
WIN = 16
B, S = 2, 8192
NCORES = 8
TOK_TOTAL = B * S
TOK_PER_CORE = TOK_TOTAL // NCORES


def _emit(nc, tokens, repeat):
    ntiles = tokens // 128

    xt = nc.dram_tensor("xt", [DIM, tokens], F32R, kind="ExternalInput")
    wqt = nc.dram_tensor("wqt", [DIM, DIM], F32R, kind="ExternalInput")
    wkt = nc.dram_tensor("wkt", [DIM, DIM], F32R, kind="ExternalInput")
    wvt = nc.dram_tensor("wvt", [DIM, DIM], F32R, kind="ExternalInput")
    wot = nc.dram_tensor("wot", [DIM, DIM], F32R, kind="ExternalInput")
    cosd = nc.dram_tensor("cosd", [128, DIM], F32, kind="ExternalInput")
    sind = nc.dram_tensor("sind", [128, DIM], F32, kind="ExternalInput")
    mskd = nc.dram_tensor("mskd", [128, 512], F32, kind="ExternalInput")
    idnd = nc.dram_tensor("idnd", [128, 128], F32, kind="ExternalInput")
    att = nc.dram_tensor("att", [DIM, tokens], F32R, kind="Internal")
    out = nc.dram_tensor("out", [tokens, DIM], F32, kind="ExternalOutput")

    xtr = xt.rearrange("(c p) t -> p c t", p=128)
    attr = att.rearrange("(c p) t -> p c t", p=128)
    wqr = wqt.rearrange("(c p) o -> p c o", p=128)
    wkr = wkt.rearrange("(c p) o -> p c o", p=128)
    wvr = wvt.rearrange("(c p) o -> p c o", p=128)
    wor = wot.rearrange("(c p) o -> p c o", p=128)

    with tile.TileContext(nc) as tc:
        with (
            tc.tile_pool(name="wpool", bufs=1) as wpool,
            tc.tile_pool(name="cpool", bufs=1) as cpool,
            tc.tile_pool(name="xpool", bufs=3) as xpool,
            tc.tile_pool(name="rpool", bufs=3) as rpool,
            tc.tile_pool(name="qkpool", bufs=2) as qkpool,
            tc.tile_pool(name="vpool", bufs=2) as vpool,
            tc.tile_pool(name="tpool", bufs=2) as tpool,
            tc.tile_pool(name="epool", bufs=3) as epool,
            tc.tile_pool(name="zpool", bufs=4) as zpool,
            tc.tile_pool(name="apool", bufs=2) as apool,
            tc.tile_pool(name="opool", bufs=2) as opool,
            tc.tile_pool(name="pp", bufs=3, space="PSUM") as pp,
            tc.tile_pool(name="ts", bufs=3, space="PSUM") as ts,
            tc.tile_pool(name="pa", bufs=2, space="PSUM") as pa,
        ):
            # Resident weights (wq/wk/wv); wo reuses wq's slot in pass 2.
            wq_sb = wpool.tile([128, 8, DIM], F32R, tag="w0")
            nc.sync.dma_start(wq_sb[:], wqr[:])
            wk_sb = wpool.tile([128, 8, DIM], F32R, tag="w1")
            nc.sync.dma_start(wk_sb[:], wkr[:])
            wv_sb = wpool.tile([128, 8, DIM], F32R, tag="w2")
            nc.sync.dma_start(wv_sb[:], wvr[:])

            cos_sb = cpool.tile([128, DIM], F32, tag="cos")
            nc.sync.dma_start(cos_sb[:], cosd[:])
            sin_sb = cpool.tile([128, DIM], F32, tag="sin")
            nc.sync.dma_start(sin_sb[:], sind[:])
            msk_sb = cpool.tile([128, 512], F32, tag="msk")
            nc.sync.dma_start(msk_sb[:], mskd[:])
            idn_sb = cpool.tile([128, 128], F32, tag="idn")
            nc.sync.dma_start(idn_sb[:], idnd[:])

            for _rep in range(repeat):
                # ---------------- pass 1: x -> attnT ----------------
                for it in range(ntiles):
                    t0 = it * 128
                    tsl = slice(t0, t0 + 128)
                    xt_t = xpool.tile([128, 8, 128], F32R, tag="xin")
                    nc.sync.dma_start(xt_t[:], xtr[:, :, tsl])

                    # Q and K projections + rope (natural layout [tok, d])
                    qr_t = qkpool.tile([128, DIM], F32, tag="qr")
                    kr_t = qkpool.tile([128, DIM], F32, tag="kr")
                    for w_sb, dst in ((wq_sb, qr_t), (wk_sb, kr_t)):
                        for hf in range(2):
                            ps = pp.tile([128, 512], F32, tag="pp")
                            for c in range(8):
                                nc.tensor.matmul(
                                    ps[:],
                                    xt_t[:, c, :],
                                    w_sb[:, c, 512 * hf : 512 * hf + 512],
                                    start=(c == 0),
                                    stop=(c == 7),
                                )
                            raw = rpool.tile([128, 512], F32, tag="praw")
                            nc.scalar.copy(raw[:], ps[:])
                            # rope: dst = raw*cos + rothalf(raw)*sin_ext
                            t1 = rpool.tile([128, 512], F32, tag="rt1")
                            rh = bass.AP(
                                tensor=raw.tensor,
                                offset=raw.offset + 32,
                                ap=[raw.ap[0], [64, 8], [-32, 2], [1, 32]],
                            )
                            sin4 = sin_sb[:, 512 * hf : 512 * hf + 512].rearrange(
                                "p (h s j) -> p h s j", h=8, s=2
                            )
                            nc.vector.tensor_mul(
                                t1[:].rearrange("p (h s j) -> p h s j", h=8, s=2),
                                rh,
                                sin4,
                            )
                            t2 = rpool.tile([128, 512], F32, tag="rt2")
                            nc.vector.tensor_mul(
                                t2[:], raw[:], cos_sb[:, 512 * hf : 512 * hf + 512]
                            )
                            nc.vector.tensor_add(
                                dst[:, 512 * hf : 512 * hf + 512], t1[:], t2[:]
                            )

                    # V projection -> vo [128, 16, 65] with ones column
                    vo_t = vpool.tile([128, NHEADS, HDIM + 1], F32, tag="vo")
                    for hf in range(2):
                        ps = pp.tile([128, 512], F32, tag="pp")
                        for c in range(8):
                            nc.tensor.matmul(
                                ps[:],
                                xt_t[:, c, :],
                                wv_sb[:, c, 512 * hf : 512 * hf + 512],
                                start=(c == 0),
                                stop=(c == 7),
                            )
                        nc.scalar.copy(
                            vo_t[:, 8 * hf : 8 * hf + 8, 0:HDIM],
                            ps[:].rearrange("p (h d) -> p h d", h=8),
                        )
                    nc.vector.memset(vo_t[:, :, HDIM : HDIM + 1], 1.0)

                    # PE transposes: Qr,Kr -> QT,KT  [d, tok]
                    qt_t = qkpool.tile([128, 8, 128], F32, tag="qt")
                    kt_t = qkpool.tile([128, 8, 128], F32, tag="kt")
                    for src, dstt in ((qr_t, qt_t), (kr_t, kt_t)):
                        for qd in range(2):
                            pt_ = ts.tile([128, 512], F32, tag="ts")
                            for c4 in range(4):
                                c = 4 * qd + c4
                                nc.tensor.transpose(
                                    pt_[:, 128 * c4 : 128 * c4 + 128],
                                    src[:, 128 * c : 128 * c + 128],
                                    idn_sb[:],
                                )
                            nc.scalar.copy(
                                dstt[:, 4 * qd : 4 * qd + 4, :],
                                pt_[:].rearrange("p (c t) -> p c t", c=4),
                            )

                    # attention per quad of 4 heads.  NOTE: matmuls writing
                    # the same PSUM bank must share one K-partition range
                    # (mixing offsets 0/64 in a bank hard-crashes the PE),
                    # so quads group heads of equal parity.
                    head_quads = [
                        [0, 2, 4, 6],
                        [8, 10, 12, 14],
                        [1, 3, 5, 7],
                        [9, 11, 13, 15],
                    ]
                    attn_t = apool.tile([128, DIM], F32, tag="attn")
                    for qd in range(4):
                        heads = head_quads[qd]
                        po = (heads[0] % 2) * 64
                        ps_s = ts.tile([128, 512], F32, tag="ts")
                        for h4, h in enumerate(heads):
                            ch = h // 2
                            nc.tensor.matmul(
                                ps_s[:, 128 * h4 : 128 * h4 + 128],
                                kt_t[po : po + 64, ch, :],
                                qt_t[po : po + 64, ch, :],
                                start=True,
                                stop=True,
                            )
                        et_t = epool.tile([128, 512], F32, tag="et")
                        nc.scalar.activation(et_t[:], ps_s[:], AF.Exp, scale=0.125)
                        nc.vector.tensor_mul(et_t[:], et_t[:], msk_sb[:])
                        pa_t = pa.tile([128, 4 * (HDIM + 1)], F32, tag="pa")
                        for h4, h in enumerate(heads):
                            nc.tensor.matmul(
                                pa_t[:, 65 * h4 : 65 * h4 + 65],
                                et_t[:, 128 * h4 : 128 * h4 + 128],
                                vo_t[:, h, :],
                                start=True,
                                stop=True,
                            )
                        ziv = zpool.tile([128, 4], F32, tag="zi")
                        zsrc = bass.AP(
                            tensor=pa_t.tensor,
                            offset=pa_t.offset + HDIM,
                            ap=[pa_t.ap[0], [HDIM + 1, 4]],
                        )
                        nc.vector.reciprocal(ziv[:], zsrc)
                        for h4, h in enumerate(heads):
                            nc.vector.tensor_scalar_mul(
                                attn_t[:, HDIM * h : HDIM * h + HDIM],
                                pa_t[:, 65 * h4 : 65 * h4 + HDIM],
                                ziv[:, h4 : h4 + 1],
                            )

                    # transpose attn -> attnT (fp32r) -> DRAM
                    at_t = tpool.tile([128, 8, 128], F32R, tag="at")
                    for qd in range(2):
                        pt_ = ts.tile([128, 512], F32, tag="ts")
                        for c4 in range(4):
                            c = 4 * qd + c4
                            nc.tensor.transpose(
                                pt_[:, 128 * c4 : 128 * c4 + 128],
                                attn_t[:, 128 * c : 128 * c + 128],
                                idn_sb[:],
                            )
                        nc.vector.tensor_copy(
                            at_t[:, 4 * qd : 4 * qd + 4, :],
                            pt_[:].rearrange("p (c t) -> p c t", c=4),
                        )
                    nc.sync.dma_start(attr[:, :, tsl], at_t[:])

                # ---------------- pass 2: attnT @ woT -> out ----------------
                wo_sb = wpool.tile([128, 8, DIM], F32R, tag="w0")
                nc.sync.dma_start(wo_sb[:], wor[:])
                for it in range(ntiles):
                    t0 = it * 128
                    tsl = slice(t0, t0 + 128)
                    at_in = xpool.tile([128, 8, 128], F32R, tag="xin")
                    nc.sync.dma_start(at_in[:], attr[:, :, tsl])
                    o_t = opool.tile([128, DIM], F32, tag="o")
                    for hf in range(2):
                        ps = pp.tile([128, 512], F32, tag="pp")
                        for c in range(8):
                            nc.tensor.matmul(
                                ps[:],
                                at_in[:, c, :],
                                wo_sb[:, c, 512 * hf : 512 * hf + 512],
                                start=(c == 0),
                                stop=(c == 7),
                            )
                        nc.scalar.copy(o_t[:, 512 * hf : 512 * hf + 512], ps[:])
                    nc.sync.dma_start(out[tsl, :], o_t[:])

                # reload wq for the next repeat (slot was taken by wo)
                if _rep + 1 < repeat:
                    wq_sb = wpool.tile([128, 8, DIM], F32R, tag="w0")
                    nc.sync.dma_start(wq_sb[:], wqr[:])
    return nc


_PROGRAMS = {}


def build_program(tokens=TOK_PER_CORE, repeat=1):
    key = (tokens, repeat)
    if key not in _PROGRAMS:
        nc = bacc.Bacc("TRN2")
        _emit(nc, tokens, repeat)
        nc.compile()
        _PROGRAMS[key] = nc
    return _PROGRAMS[key]


def host_tables(rope_freqs):
    freqs = np.asarray(rope_freqs, dtype=np.float32)[:WIN]  # [16, 32]
    cos = np.cos(freqs)
    sin = np.sin(freqs)
    pos = np.arange(128) % WIN
    cos_ext = np.zeros((128, DIM), dtype=np.float32)
    sin_ext = np.zeros((128, DIM), dtype=np.float32)
    for h in range(NHEADS):
        cos_ext[:, h * 64 + 0 : h * 64 + 32] = cos[pos]
        cos_ext[:, h * 64 + 32 : h * 64 + 64] = cos[pos]
        sin_ext[:, h * 64 + 0 : h * 64 + 32] = -sin[pos]
        sin_ext[:, h * 64 + 32 : h * 64 + 64] = sin[pos]
    p = np.arange(128)
    c = np.arange(128)
    blk = (p[:, None] // WIN == c[None, :] // WIN).astype(np.float32)
    msk = np.tile(blk, (1, 4))
    idn = np.eye(128, dtype=np.float32)
    return cos_ext, sin_ext, np.ascontiguousarray(msk), idn


def make_in_maps(x, rope_freqs, wq, wk, wv, wo, tokens=TOK_PER_CORE, ncores=NCORES):
    x = np.asarray(x, dtype=np.float32)
    xf = x.reshape(-1, DIM)
    xT = np.ascontiguousarray(xf.T)  # [DIM, TOK_TOTAL]
    wqt = np.ascontiguousarray(np.asarray(wq, dtype=np.float32).T)
    wkt = np.ascontiguousarray(np.asarray(wk, dtype=np.float32).T)
    wvt = np.ascontiguousarray(np.asarray(wv, dtype=np.float32).T)
    wot = np.ascontiguousarray(np.asarray(wo, dtype=np.float32).T)
    cos_ext, sin_ext, msk, idn = host_tables(rope_freqs)
    maps = []
    for c in range(ncores):
        sl = slice(c * tokens, (c + 1) * tokens)
        maps.append(
            {
                "xt": np.ascontiguousarray(xT[:, sl]),
                "wqt": wqt,
                "wkt": wkt,
                "wvt": wvt,
                "wot": wot,
                "cosd": cos_ext,
                "sind": sin_ext,
                "mskd": msk,
                "idnd": idn,
            }
        )
    return maps


def kernel(x, rope_freqs, wq, wk, wv, wo):
    nc = build_program(TOK_PER_CORE, 1)
    maps = make_in_maps(x, rope_freqs, wq, wk, wv, wo)
    res = run_bass_kernel_spmd(nc, maps, core_ids=list(range(NCORES)))
    outs = [res.results[c]["out"] for c in range(NCORES)]
    full = np.concatenate(outs, axis=0)  # [TOK_TOTAL, DIM]
    return full.reshape(B, S, DIM).astype(np.float32)
